# revision 6
# baseline (speedup 1.0000x reference)
"""Distributed SimCLR/NT-Xent contrastive loss on 8 Trainium2 NeuronCores.

Symmetric-halving strategy: sim = sim^T, so each element's exp contributes to
exactly two row sums (rows i and j). Each core owns 2048 rows (inputs rotated
by -core*2048 so its rows are global columns [0, 2048)) and computes only the
upper-triangular part of its strip at 128x128 subblock granularity:

  range A: columns [m*128, 8192)          (own diag block tri + blocks 1..3)
  range B: columns [8192 + m*128, 10240)  (block 4 tri, split with core+4)

for each 128-row strip m. Row sums come from the ACT Exp pass (fused affine
exp(S*dot - S), S = 1/temperature, fixed shift since rows are unit norm);
the mirrored contributions are recovered as column sums of each exp'd
128-column chunk via near-free tiny matmuls (lhsT = exp chunk, rhs = ones,
moving dim 1). Blocks 5..7 are never touched: their contributions arrive as
column sums computed by cores me+5..7. The final assembly (adding column-sum
contributions into other rows' denominators, log, mean) happens on the host
in float64.

loss = 1/T + mean_i log(S_i) - mean_i dot(a_i, b_i)/T
"""

import sys

if "/opt/trn_rl_repo" not in sys.path:
    sys.path.insert(0, "/opt/trn_rl_repo")

import numpy as np

import concourse.bass as bass
import concourse.mybir as mybir
from concourse import masks
from concourse.tile import TileContext
from concourse.bass_utils import run_bass_kernel_spmd

# ---------------------------------------------------------------------------
# Compatibility patches for the walrus build in this container:
#  * EVENT_SEMAPHORE_RANGE_CLEAR fails codegen ("ISA wrong length"), and
#  * the tile teardown Drain carries >2 sem waits ("Too many sync wait
#    commands").
# Replace the teardown with per-proc single-wait drains + barriers and skip
# the on-device semaphore clear (allocator bookkeeping is kept).
# ---------------------------------------------------------------------------


def _patched_clear_and_free_semaphores(self, sems):
    if not sems:
        return
    sem_nums = [
        s.num if isinstance(s, bass.SemaphoreHandle) else s for s in sems
    ]
    self._state.prepend_free_semaphores(sem_nums)
    for poison_set in self._tile_sem_poison_stack:
        poison_set.update(sem_nums)


def _patched_drain_and_barrier(self, tick_clock, wait_clock):
    nc = self.nc
    clock = tick_clock.global_clock
    assert self.sems is not None
    allocated = self.sems.allocated()  # proc index -> SemaphoreHandle
    for proc in sorted(allocated):
        sem = allocated[proc]
        tick = clock[proc]
        if tick <= 0:
            continue
        mult = 16 if sem.name.startswith("DMA") else 1
        d = nc.sync.drain()
        d.wait_op(sem, tick * mult, "sem-ge")
    nc.all_engine_barrier()
    popped = nc._tile_sem_poison_stack.pop()
    assert popped is self._sem_poison
    nc.clear_and_free_semaphores(list(allocated.values()))
    nc.all_engine_barrier()


bass.Bass.clear_and_free_semaphores = _patched_clear_and_free_semaphores
TileContext._drain_and_barrier = _patched_drain_and_barrier


def _hoist_excess_waits(nc, limit=1):
    """This walrus supports only `limit` sync waits per instruction. Hoist
    the excess onto standalone EventSemaphore instructions inserted just
    before the over-subscribed instruction on the same engine (per-engine
    program order makes this semantically identical)."""
    import bass_rust

    counter = 0
    for bb in nc.main_func.blocks:
        insts = bb.instructions
        new = []
        changed = False
        for ins in insts:
            si = ins.sync_info
            if si is not None:
                waits = list(si.on_wait)
                if len(waits) > limit:
                    excess, keep = waits[:-limit], waits[-limit:]
                    for w in excess:
                        counter += 1
                        ev = mybir.InstEventSemaphore(
                            name=f"hoistw-{counter}",
                            engine=ins.engine,
                            ins=[],
                            outs=[],
                        )
                        ev.sync_info = bass_rust.SyncInfo(
                            on_wait=[w], on_update=[]
                        )
                        new.append(ev)
                    ins.sync_info = bass_rust.SyncInfo(
                        on_wait=keep, on_update=list(si.on_update)
                    )
                    changed = True
            new.append(ins)
        if changed:
            bb.instructions = new

TEMPERATURE = 0.07
B, D = 8192, 128
N2 = 2 * B
NCORES = 8
P = 128
RPC = N2 // NCORES      # 2048 rows per core
MT = RPC // P           # 16 row strips
BS = 2048               # block size (columns per block / rows per DMA block)
JB = BS // P            # 16 rows packed per partition
NBLK = 5                # et blocks needed: 0..4
PW = 1536               # psum piece width (3 banks)
NSLOT = 78              # column-sum slots: b0 15 + b1..3 48 + b4 15
MAXPIECES = 8           # per-strip piece slots in partials

F32 = mybir.dt.float32
BF16 = mybir.dt.bfloat16
AF = mybir.ActivationFunctionType
ALU = mybir.AluOpType
AX = mybir.AxisListType


def _slot_of(k, cc):
    """csacc slot for colsums of block k, chunk cc (None = no colsum)."""
    if k == 0:
        return cc - 1           # cc in 1..15 -> 0..14
    if k in (1, 2, 3):
        return 15 + (k - 1) * 16 + cc   # 15..62
    return 63 + cc - 1          # block 4, cc in 1..15 -> 63..77


def _strip_pieces(m):
    """Pieces for strip m: list of (g0, w, mask_diag, s0, slot_chunks)
    where slot_chunks = list of (chunk_index_within_piece, slot)."""
    pieces = []
    for lo, hi in ((m * 128, 8192), (8192 + m * 128, 10240)):
        pos = lo
        while pos < hi:
            w = min(PW, hi - pos)
            slot_chunks = []
            mask_diag = False
            for ci in range(w // 128):
                c_abs = (pos + ci * 128) // 128
                k, cc = c_abs // 16, c_abs % 16
                if cc == m and k in (0, 4):
                    if k == 0:
                        mask_diag = True
                        assert ci == 0
                    continue
                slot_chunks.append((ci, _slot_of(k, cc)))
            if slot_chunks:
                s0 = slot_chunks[0][1]
                assert [s for _, s in slot_chunks] == list(
                    range(s0, s0 + len(slot_chunks))
                )
            pieces.append((pos, w, mask_diag, slot_chunks))
            pos += w
    assert len(pieces) <= MAXPIECES
    return pieces


def _build_bass(hoist=True):
    scale = 1.0 / TEMPERATURE

    nc = bass.Bass()
    allx = nc.dram_tensor("allx", [NBLK * BS, D], BF16, kind="ExternalInput")
    out = nc.dram_tensor("out", [P, 128 + NSLOT + JB], F32,
                         kind="ExternalOutput")

    # packed view: row = b*BS + p*JB + j
    allx_b = allx[:].rearrange("(b p j) d -> b p (j d)", p=P, j=JB)

    with TileContext(nc) as tc:
        with (
            tc.tile_pool(name="persist", bufs=1) as persist,
            tc.tile_pool(name="raw0", bufs=1) as raw0_pool,
            tc.tile_pool(name="rawx", bufs=3) as rawx_pool,
            tc.tile_pool(name="xn", bufs=2) as xn_pool,
            tc.tile_pool(name="sq", bufs=2) as sq_pool,
            tc.tile_pool(name="exps", bufs=4) as exps_pool,
            tc.tile_pool(name="psum", bufs=2, space="PSUM") as psum_pool,
            tc.tile_pool(name="cspsum", bufs=2, space="PSUM") as cs_pool,
        ):
            ident = persist.tile([P, P], BF16, tag="ident")
            masks.make_identity(nc, ident[:])
            bias_negs = persist.tile([P, 1], F32, tag="bias_negs")
            nc.gpsimd.memset(bias_negs[:], -scale)
            ones_bf = persist.tile([P, 1], BF16, tag="ones_bf")
            nc.gpsimd.memset(ones_bf[:], 1.0)
            dmask = persist.tile([P, P], F32, tag="dmask")
            nc.gpsimd.memset(dmask[:], 0.0)
            nc.gpsimd.affine_select(
                out=dmask[:], in_=dmask[:],
                compare_op=ALU.not_equal, fill=-1.0e9,
                base=0, pattern=[[-1, P]], channel_multiplier=1,
            )

            et = persist.tile([P, NBLK * BS], BF16, tag="et")
            norms2 = persist.tile([P, NBLK * JB], F32, tag="norms2")
            rsq = persist.tile([P, NBLK * JB], F32, tag="rsq")
            lntmp = persist.tile([P, NBLK * JB], F32, tag="lntmp")
            rawdot = persist.tile([P, JB], F32, tag="rawdot")
            pos2 = persist.tile([P, JB], F32, tag="pos2")
            pospart = persist.tile([P, JB], F32, tag="pospart")
            partials = persist.tile([P, MT * MAXPIECES], F32, tag="partials")
            csacc = persist.tile([P, NSLOT], F32, tag="csacc")
            nc.gpsimd.memset(csacc[:], 0.0)

            # ---- build ET (normalized, transposed, bf16), blocks 0..4 ----
            raw_blocks = {}
            for b in range(NBLK):
                pool = raw0_pool if b == 0 else rawx_pool
                rx = pool.tile([P, BS], BF16, tag="raw0" if b == 0 else "")
                nc.sync.dma_start(rx[:], allx_b[b])
                raw_blocks[b] = rx
                rx3 = rx[:].rearrange("p (j d) -> p j d", d=D)
                js = slice(b * JB, (b + 1) * JB)
                sq = sq_pool.tile([P, BS], F32)
                nc.gpsimd.tensor_mul(sq[:], rx[:], rx[:])
                nc.vector.reduce_sum(
                    norms2[:, js], sq[:].rearrange("p (j d) -> p j d", d=D),
                    axis=AX.X,
                )
                # rsqrt(x) = exp(-0.5*ln(x)); Ln+Exp share one table set
                nc.scalar.activation(lntmp[:, js], norms2[:, js], AF.Ln)
                nc.scalar.activation(rsq[:, js], lntmp[:, js], AF.Exp,
                                     scale=-0.5)
                xn = xn_pool.tile([P, BS], BF16)
                nc.vector.tensor_mul(
                    xn[:].rearrange("p (j d) -> p j d", d=D),
                    rx3,
                    rsq[:, js].to_broadcast((P, JB, D)),
                )
                xn3 = xn[:].rearrange("p (j d) -> p j d", d=D)
                # transpose the 16 row-groups; 12 into ps1, 4 into ps2
                etb = et[:, b * BS:(b + 1) * BS].rearrange(
                    "q (p j) -> q p j", j=JB
                )
                ps1 = psum_pool.tile([P, PW], BF16, tag="ps")
                for j in range(12):
                    nc.tensor.transpose(
                        ps1[:, j * P:(j + 1) * P], xn3[:, j, :], ident[:]
                    )
                nc.vector.tensor_copy(
                    etb[:, :, 0:12],
                    ps1[:].rearrange("q (j p) -> q p j", p=P),
                )
                ps2 = psum_pool.tile([P, PW], BF16, tag="ps")
                for j in range(4):
                    nc.tensor.transpose(
                        ps2[:, j * P:(j + 1) * P], xn3[:, 12 + j, :], ident[:]
                    )
                nc.vector.tensor_copy(
                    etb[:, :, 12:16],
                    ps2[:, 0:512].rearrange("q (j p) -> q p j", p=P),
                )
                if b == 4:
                    # positive-pair raw dots: my rows x partner rows
                    r0 = raw_blocks[0][:].rearrange("p (j d) -> p j d", d=D)
                    rp = raw_blocks[4][:].rearrange("p (j d) -> p j d", d=D)
                    pd = sq_pool.tile([P, BS], F32)
                    pd3 = pd[:].rearrange("p (j d) -> p j d", d=D)
                    nc.vector.tensor_mul(pd3[:], r0[:], rp[:])
                    nc.vector.reduce_sum(rawdot[:], pd3[:], axis=AX.X)
                    nc.vector.tensor_mul(pos2[:], rawdot[:], rsq[:, 0:JB])
                    nc.vector.tensor_mul(
                        pospart[:], pos2[:], rsq[:, 4 * JB:5 * JB]
                    )

            # ---- main loop: upper-tri pieces, exp row-sums, col-sums ----
            # Software-pipelined with LAG: the tiny column-sum matmuls for
            # piece i (which wait on exp(i)) are emitted after the main
            # matmuls of piece i+LAG, so PE's in-order stream never stalls
            # waiting for ACT.
            LAG = 2
            all_pieces = [
                (m, pi) + pc
                for m in range(MT)
                for pi, pc in enumerate(_strip_pieces(m))
            ]

            def emit_colsums(piece, ex):
                _m, _pi, _g0, _w, _md, slot_chunks = piece
                if not slot_chunks:
                    return
                cs = cs_pool.tile([P, JB], F32)
                s0 = slot_chunks[0][1]
                n = len(slot_chunks)
                for si, (ci, _slot) in enumerate(slot_chunks):
                    nc.tensor.matmul(
                        cs[:, si:si + 1],
                        ex[:, ci * P:(ci + 1) * P],
                        ones_bf[:],
                        start=True, stop=True,
                    )
                nc.vector.tensor_add(
                    csacc[:, s0:s0 + n], csacc[:, s0:s0 + n], cs[:, 0:n]
                )

            inflight = []
            for piece in all_pieces:
                m, pi, g0, w, mask_diag, slot_chunks = piece
                lhsT = et[:, m * P:(m + 1) * P]
                ps = psum_pool.tile([P, PW], F32, tag="ps")
                # Lead with a short matmul: the first instruction of a PE
                # burst issues at the lowest p-state, so keep it to 128
                # columns and let the rest run at the mid p-state.
                cuts = [0, 128] + list(range(512, w, 512)) + [w]
                for c0, c1 in zip(cuts, cuts[1:]):
                    if c0 >= c1:
                        continue
                    nc.tensor.matmul(
                        ps[:, c0:c1], lhsT, et[:, g0 + c0:g0 + c1],
                        start=True, stop=True,
                    )
                if mask_diag:
                    nc.vector.tensor_add(ps[:, 0:P], ps[:, 0:P], dmask[:])
                ex = exps_pool.tile([P, PW], BF16, name="ex")
                pidx = m * MAXPIECES + pi
                nc.scalar.activation(
                    ex[:, 0:w], ps[:, 0:w], AF.Exp,
                    bias=bias_negs[:], scale=scale,
                    accum_out=partials[:, pidx:pidx + 1],
                )
                inflight.append((piece, ex))
                if len(inflight) > LAG:
                    emit_colsums(*inflight.pop(0))
            for pc_ex in inflight:
                emit_colsums(*pc_ex)

            # ---- outputs: raw partials, colsums, positive dots ----
            nc.sync.dma_start(out[:, 0:128], partials[:])
            nc.sync.dma_start(out[:, 128:128 + NSLOT], csacc[:])
            nc.sync.dma_start(out[:, 128 + NSLOT:], pospart[:])

    if hoist:
        _hoist_excess_waits(nc, limit=1)
    return nc


def _in_maps(embeddings_a, embeddings_b, ncores=NCORES):
    import ml_dtypes

    allx = np.ascontiguousarray(
        np.concatenate([embeddings_a, embeddings_b], axis=0)
    ).astype(ml_dtypes.bfloat16)
    maps = []
    for c in range(ncores):
        # rotate so this core's rows sit at columns [0, RPC); only blocks
        # 0..4 are needed (5..7 arrive as other cores' column sums)
        rot = np.ascontiguousarray(
            np.roll(allx, -c * RPC, axis=0)[: NBLK * BS]
        )
        maps.append({"allx": rot})
    return maps


def _combine(outs):
    """outs: per-core [P, 128+NSLOT+JB] raw partials -> scalar loss."""
    S = np.zeros(N2, dtype=np.float64)
    dots = 0.0
    prow = np.arange(P)
    for c, o in enumerate(outs):
        o64 = np.asarray(o, dtype=np.float64)
        r0 = c * RPC
        part = o64[:, 0:128]
        cso = o64[:, 128:128 + NSLOT]
        pos = o64[:, 128 + NSLOT:]
        # row-accum partial sums per strip
        for m in range(MT):
            npc = len(_strip_pieces(m))
            rows = (r0 + m * 128 + prow) % N2
            S[rows] += part[:, m * MAXPIECES:m * MAXPIECES + npc].sum(axis=1)
        # column-sum contributions
        for s in range(NSLOT):
            if s < 15:
                k, cc = 0, s + 1
            elif s < 63:
                k, cc = 1 + (s - 15) // 16, (s - 15) % 16
            else:
                k, cc = 4, s - 63 + 1
            rows = (r0 + k * BS + cc * 128 + prow) % N2
            S[rows] += cso[:, s]
        dots += pos.sum()
    inv_t = 1.0 / TEMPERATURE
    loss = inv_t + np.mean(np.log(S)) - (dots * inv_t) / N2
    return np.float32(loss)


_NC_CACHE = {}


def _get_nc():
    if "nc" not in _NC_CACHE:
        _NC_CACHE["nc"] = _build_bass()
    return _NC_CACHE["nc"]


def kernel(embeddings_a, embeddings_b):
    nc = _get_nc()
    maps = _in_maps(embeddings_a, embeddings_b)
    res = run_bass_kernel_spmd(nc, maps, list(range(NCORES)), trace=False)
    return _combine([r["out"] for r in res.results])


# revision 7
# speedup vs baseline: 1.0173x; 1.0173x over previous
"""Distributed SimCLR/NT-Xent contrastive loss on 8 Trainium2 NeuronCores.

Symmetric-halving strategy: sim = sim^T, so each element's exp contributes to
exactly two row sums (rows i and j). Each core owns 2048 rows (inputs rotated
by -core*2048 so its rows are global columns [0, 2048)) and computes only the
upper-triangular part of its strip at 128x128 subblock granularity:

  range A: columns [m*128, 8192)          (own diag block tri + blocks 1..3)
  range B: columns [8192 + m*128, 10240)  (block 4 tri, split with core+4)

for each 128-row strip m. Row sums come from the ACT Exp pass (fused affine
exp(S*dot - S), S = 1/temperature, fixed shift since rows are unit norm);
the mirrored contributions are recovered as column sums of each exp'd
128-column chunk via near-free tiny matmuls (lhsT = exp chunk, rhs = ones,
moving dim 1). Blocks 5..7 are never touched: their contributions arrive as
column sums computed by cores me+5..7. The final assembly (adding column-sum
contributions into other rows' denominators, log, mean) happens on the host
in float64.

loss = 1/T + mean_i log(S_i) - mean_i dot(a_i, b_i)/T
"""

import sys

if "/opt/trn_rl_repo" not in sys.path:
    sys.path.insert(0, "/opt/trn_rl_repo")

import numpy as np

import concourse.bass as bass
import concourse.mybir as mybir
from concourse import masks
from concourse.tile import TileContext
from concourse.bass_utils import run_bass_kernel_spmd

# ---------------------------------------------------------------------------
# Compatibility patches for the walrus build in this container:
#  * EVENT_SEMAPHORE_RANGE_CLEAR fails codegen ("ISA wrong length"), and
#  * the tile teardown Drain carries >2 sem waits ("Too many sync wait
#    commands").
# Replace the teardown with per-proc single-wait drains + barriers and skip
# the on-device semaphore clear (allocator bookkeeping is kept).
# ---------------------------------------------------------------------------


def _patched_clear_and_free_semaphores(self, sems):
    if not sems:
        return
    sem_nums = [
        s.num if isinstance(s, bass.SemaphoreHandle) else s for s in sems
    ]
    self._state.prepend_free_semaphores(sem_nums)
    for poison_set in self._tile_sem_poison_stack:
        poison_set.update(sem_nums)


def _patched_drain_and_barrier(self, tick_clock, wait_clock):
    nc = self.nc
    clock = tick_clock.global_clock
    assert self.sems is not None
    allocated = self.sems.allocated()  # proc index -> SemaphoreHandle
    for proc in sorted(allocated):
        sem = allocated[proc]
        tick = clock[proc]
        if tick <= 0:
            continue
        mult = 16 if sem.name.startswith("DMA") else 1
        d = nc.sync.drain()
        d.wait_op(sem, tick * mult, "sem-ge")
    nc.all_engine_barrier()
    popped = nc._tile_sem_poison_stack.pop()
    assert popped is self._sem_poison
    nc.clear_and_free_semaphores(list(allocated.values()))
    nc.all_engine_barrier()


bass.Bass.clear_and_free_semaphores = _patched_clear_and_free_semaphores
TileContext._drain_and_barrier = _patched_drain_and_barrier


def _hoist_excess_waits(nc, limit=1):
    """This walrus supports only `limit` sync waits per instruction. Hoist
    the excess onto standalone EventSemaphore instructions inserted just
    before the over-subscribed instruction on the same engine (per-engine
    program order makes this semantically identical)."""
    import bass_rust

    counter = 0
    for bb in nc.main_func.blocks:
        insts = bb.instructions
        new = []
        changed = False
        for ins in insts:
            si = ins.sync_info
            if si is not None:
                waits = list(si.on_wait)
                if len(waits) > limit:
                    excess, keep = waits[:-limit], waits[-limit:]
                    for w in excess:
                        counter += 1
                        ev = mybir.InstEventSemaphore(
                            name=f"hoistw-{counter}",
                            engine=ins.engine,
                            ins=[],
                            outs=[],
                        )
                        ev.sync_info = bass_rust.SyncInfo(
                            on_wait=[w], on_update=[]
                        )
                        new.append(ev)
                    ins.sync_info = bass_rust.SyncInfo(
                        on_wait=keep, on_update=list(si.on_update)
                    )
                    changed = True
            new.append(ins)
        if changed:
            bb.instructions = new

TEMPERATURE = 0.07
B, D = 8192, 128
N2 = 2 * B
NCORES = 8
P = 128
RPC = N2 // NCORES      # 2048 rows per core
MT = RPC // P           # 16 row strips
BS = 2048               # block size (columns per block / rows per DMA block)
JB = BS // P            # 16 rows packed per partition
NBLK = 5                # et blocks needed: 0..4
PW = 1536               # psum piece width (3 banks)
NSLOT = 78              # column-sum slots: b0 15 + b1..3 48 + b4 15
MAXPIECES = 8           # per-strip piece slots in partials

F32 = mybir.dt.float32
BF16 = mybir.dt.bfloat16
AF = mybir.ActivationFunctionType
ALU = mybir.AluOpType
AX = mybir.AxisListType


def _slot_of(k, cc):
    """csacc slot for colsums of block k, chunk cc (None = no colsum)."""
    if k == 0:
        return cc - 1           # cc in 1..15 -> 0..14
    if k in (1, 2, 3):
        return 15 + (k - 1) * 16 + cc   # 15..62
    return 63 + cc - 1          # block 4, cc in 1..15 -> 63..77


def _strip_pieces(m):
    """Pieces for strip m: list of (g0, w, mask_diag, s0, slot_chunks)
    where slot_chunks = list of (chunk_index_within_piece, slot)."""
    pieces = []
    for lo, hi in ((m * 128, 8192), (8192 + m * 128, 10240)):
        pos = lo
        while pos < hi:
            w = min(PW, hi - pos)
            slot_chunks = []
            mask_diag = False
            for ci in range(w // 128):
                c_abs = (pos + ci * 128) // 128
                k, cc = c_abs // 16, c_abs % 16
                if cc == m and k in (0, 4):
                    if k == 0:
                        mask_diag = True
                        assert ci == 0
                    continue
                slot_chunks.append((ci, _slot_of(k, cc)))
            if slot_chunks:
                s0 = slot_chunks[0][1]
                assert [s for _, s in slot_chunks] == list(
                    range(s0, s0 + len(slot_chunks))
                )
            pieces.append((pos, w, mask_diag, slot_chunks))
            pos += w
    assert len(pieces) <= MAXPIECES
    return pieces


def _build_bass(hoist=True):
    scale = 1.0 / TEMPERATURE

    nc = bass.Bass()
    allx = nc.dram_tensor("allx", [NBLK * BS, D], BF16, kind="ExternalInput")
    out = nc.dram_tensor("out", [P, 128 + NSLOT + JB], F32,
                         kind="ExternalOutput")

    # packed view: row = b*BS + p*JB + j
    allx_b = allx[:].rearrange("(b p j) d -> b p (j d)", p=P, j=JB)

    with TileContext(nc) as tc:
        with (
            tc.tile_pool(name="persist", bufs=1) as persist,
            tc.tile_pool(name="raw0", bufs=1) as raw0_pool,
            tc.tile_pool(name="rawx", bufs=3) as rawx_pool,
            tc.tile_pool(name="xn", bufs=2) as xn_pool,
            tc.tile_pool(name="sq", bufs=2) as sq_pool,
            tc.tile_pool(name="exps", bufs=4) as exps_pool,
            tc.tile_pool(name="psum", bufs=2, space="PSUM") as psum_pool,
            tc.tile_pool(name="cspsum", bufs=2, space="PSUM") as cs_pool,
        ):
            ident = persist.tile([P, P], BF16, tag="ident")
            masks.make_identity(nc, ident[:])
            bias_negs = persist.tile([P, 1], F32, tag="bias_negs")
            nc.gpsimd.memset(bias_negs[:], -scale)
            ones_bf = persist.tile([P, 1], BF16, tag="ones_bf")
            nc.gpsimd.memset(ones_bf[:], 1.0)
            dmask = persist.tile([P, P], F32, tag="dmask")
            nc.gpsimd.memset(dmask[:], 0.0)
            nc.gpsimd.affine_select(
                out=dmask[:], in_=dmask[:],
                compare_op=ALU.not_equal, fill=-1.0e9,
                base=0, pattern=[[-1, P]], channel_multiplier=1,
            )

            et = persist.tile([P, NBLK * BS], BF16, tag="et")
            norms2 = persist.tile([P, NBLK * JB], F32, tag="norms2")
            rsq = persist.tile([P, NBLK * JB], F32, tag="rsq")
            lntmp = persist.tile([P, NBLK * JB], F32, tag="lntmp")
            rawdot = persist.tile([P, JB], F32, tag="rawdot")
            pos2 = persist.tile([P, JB], F32, tag="pos2")
            pospart = persist.tile([P, JB], F32, tag="pospart")
            partials = persist.tile([P, MT * MAXPIECES], F32, tag="partials")
            csacc = persist.tile([P, NSLOT], F32, tag="csacc")
            nc.gpsimd.memset(csacc[:], 0.0)

            # ---- build ET (normalized, transposed, bf16), blocks 0..4 ----
            raw_blocks = {}
            for b in range(NBLK):
                pool = raw0_pool if b == 0 else rawx_pool
                rx = pool.tile([P, BS], BF16, tag="raw0" if b == 0 else "")
                nc.sync.dma_start(rx[:], allx_b[b])
                raw_blocks[b] = rx
                rx3 = rx[:].rearrange("p (j d) -> p j d", d=D)
                js = slice(b * JB, (b + 1) * JB)
                sq = sq_pool.tile([P, BS], F32)
                nc.gpsimd.tensor_mul(sq[:], rx[:], rx[:])
                nc.vector.reduce_sum(
                    norms2[:, js], sq[:].rearrange("p (j d) -> p j d", d=D),
                    axis=AX.X,
                )
                # rsqrt(x) = exp(-0.5*ln(x)); Ln+Exp share one table set
                nc.scalar.activation(lntmp[:, js], norms2[:, js], AF.Ln)
                nc.scalar.activation(rsq[:, js], lntmp[:, js], AF.Exp,
                                     scale=-0.5)
                xn = xn_pool.tile([P, BS], BF16)
                nc.vector.tensor_mul(
                    xn[:].rearrange("p (j d) -> p j d", d=D),
                    rx3,
                    rsq[:, js].to_broadcast((P, JB, D)),
                )
                xn3 = xn[:].rearrange("p (j d) -> p j d", d=D)
                # transpose the 16 row-groups; 12 into ps1, 4 into ps2
                etb = et[:, b * BS:(b + 1) * BS].rearrange(
                    "q (p j) -> q p j", j=JB
                )
                ps1 = psum_pool.tile([P, PW], BF16, tag="ps")
                for j in range(12):
                    nc.tensor.transpose(
                        ps1[:, j * P:(j + 1) * P], xn3[:, j, :], ident[:]
                    )
                nc.vector.tensor_copy(
                    etb[:, :, 0:12],
                    ps1[:].rearrange("q (j p) -> q p j", p=P),
                )
                ps2 = psum_pool.tile([P, PW], BF16, tag="ps")
                for j in range(4):
                    nc.tensor.transpose(
                        ps2[:, j * P:(j + 1) * P], xn3[:, 12 + j, :], ident[:]
                    )
                nc.vector.tensor_copy(
                    etb[:, :, 12:16],
                    ps2[:, 0:512].rearrange("q (j p) -> q p j", p=P),
                )
                if b == 4:
                    # positive-pair raw dots: my rows x partner rows
                    r0 = raw_blocks[0][:].rearrange("p (j d) -> p j d", d=D)
                    rp = raw_blocks[4][:].rearrange("p (j d) -> p j d", d=D)
                    pd = sq_pool.tile([P, BS], F32)
                    pd3 = pd[:].rearrange("p (j d) -> p j d", d=D)
                    nc.vector.tensor_mul(pd3[:], r0[:], rp[:])
                    nc.vector.reduce_sum(rawdot[:], pd3[:], axis=AX.X)
                    nc.vector.tensor_mul(pos2[:], rawdot[:], rsq[:, 0:JB])
                    nc.vector.tensor_mul(
                        pospart[:], pos2[:], rsq[:, 4 * JB:5 * JB]
                    )

            # ---- main loop: upper-tri pieces, exp row-sums, col-sums ----
            # Software-pipelined with LAG: the tiny column-sum matmuls for
            # piece i (which wait on exp(i)) are emitted after the main
            # matmuls of piece i+LAG, so PE's in-order stream never stalls
            # waiting for ACT.
            LAG = 2
            # Piece-major order: all strips' piece 0 first (they only need
            # ET blocks 0-1), then piece 1, ... so the first exp wave starts
            # while ET blocks 2-4 are still being built.
            per_strip = [_strip_pieces(m) for m in range(MT)]
            all_pieces = [
                (m, pi) + per_strip[m][pi]
                for pi in range(max(len(p) for p in per_strip))
                for m in range(MT)
                if pi < len(per_strip[m])
            ]

            def emit_colsums(piece, ex):
                _m, _pi, _g0, _w, _md, slot_chunks = piece
                if not slot_chunks:
                    return
                cs = cs_pool.tile([P, JB], F32)
                s0 = slot_chunks[0][1]
                n = len(slot_chunks)
                for si, (ci, _slot) in enumerate(slot_chunks):
                    nc.tensor.matmul(
                        cs[:, si:si + 1],
                        ex[:, ci * P:(ci + 1) * P],
                        ones_bf[:],
                        start=True, stop=True,
                    )
                nc.vector.tensor_add(
                    csacc[:, s0:s0 + n], csacc[:, s0:s0 + n], cs[:, 0:n]
                )

            inflight = []
            for piece in all_pieces:
                m, pi, g0, w, mask_diag, slot_chunks = piece
                lhsT = et[:, m * P:(m + 1) * P]
                ps = psum_pool.tile([P, PW], F32, tag="ps")
                # Lead with a short matmul: the first instruction of a PE
                # burst issues at the lowest p-state, so keep it to 128
                # columns and let the rest run at the mid p-state.
                cuts = [0, 128] + list(range(512, w, 512)) + [w]
                for c0, c1 in zip(cuts, cuts[1:]):
                    if c0 >= c1:
                        continue
                    nc.tensor.matmul(
                        ps[:, c0:c1], lhsT, et[:, g0 + c0:g0 + c1],
                        start=True, stop=True,
                    )
                if mask_diag:
                    nc.vector.tensor_add(ps[:, 0:P], ps[:, 0:P], dmask[:])
                ex = exps_pool.tile([P, PW], BF16, name="ex")
                pidx = m * MAXPIECES + pi
                nc.scalar.activation(
                    ex[:, 0:w], ps[:, 0:w], AF.Exp,
                    bias=bias_negs[:], scale=scale,
                    accum_out=partials[:, pidx:pidx + 1],
                )
                inflight.append((piece, ex))
                if len(inflight) > LAG:
                    emit_colsums(*inflight.pop(0))
            for pc_ex in inflight:
                emit_colsums(*pc_ex)

            # ---- outputs: raw partials, colsums, positive dots ----
            nc.sync.dma_start(out[:, 0:128], partials[:])
            nc.sync.dma_start(out[:, 128:128 + NSLOT], csacc[:])
            nc.sync.dma_start(out[:, 128 + NSLOT:], pospart[:])

    if hoist:
        _hoist_excess_waits(nc, limit=1)
    return nc


def _in_maps(embeddings_a, embeddings_b, ncores=NCORES):
    import ml_dtypes

    allx = np.ascontiguousarray(
        np.concatenate([embeddings_a, embeddings_b], axis=0)
    ).astype(ml_dtypes.bfloat16)
    maps = []
    for c in range(ncores):
        # rotate so this core's rows sit at columns [0, RPC); only blocks
        # 0..4 are needed (5..7 arrive as other cores' column sums)
        rot = np.ascontiguousarray(
            np.roll(allx, -c * RPC, axis=0)[: NBLK * BS]
        )
        maps.append({"allx": rot})
    return maps


def _combine(outs):
    """outs: per-core [P, 128+NSLOT+JB] raw partials -> scalar loss."""
    S = np.zeros(N2, dtype=np.float64)
    dots = 0.0
    prow = np.arange(P)
    for c, o in enumerate(outs):
        o64 = np.asarray(o, dtype=np.float64)
        r0 = c * RPC
        part = o64[:, 0:128]
        cso = o64[:, 128:128 + NSLOT]
        pos = o64[:, 128 + NSLOT:]
        # row-accum partial sums per strip
        for m in range(MT):
            npc = len(_strip_pieces(m))
            rows = (r0 + m * 128 + prow) % N2
            S[rows] += part[:, m * MAXPIECES:m * MAXPIECES + npc].sum(axis=1)
        # column-sum contributions
        for s in range(NSLOT):
            if s < 15:
                k, cc = 0, s + 1
            elif s < 63:
                k, cc = 1 + (s - 15) // 16, (s - 15) % 16
            else:
                k, cc = 4, s - 63 + 1
            rows = (r0 + k * BS + cc * 128 + prow) % N2
            S[rows] += cso[:, s]
        dots += pos.sum()
    inv_t = 1.0 / TEMPERATURE
    loss = inv_t + np.mean(np.log(S)) - (dots * inv_t) / N2
    return np.float32(loss)


_NC_CACHE = {}


def _get_nc():
    if "nc" not in _NC_CACHE:
        _NC_CACHE["nc"] = _build_bass()
    return _NC_CACHE["nc"]


def kernel(embeddings_a, embeddings_b):
    nc = _get_nc()
    maps = _in_maps(embeddings_a, embeddings_b)
    res = run_bass_kernel_spmd(nc, maps, list(range(NCORES)), trace=False)
    return _combine([r["out"] for r in res.results])


# revision 11
# speedup vs baseline: 1.0878x; 1.0693x over previous
"""Distributed SimCLR/NT-Xent contrastive loss on 8 Trainium2 NeuronCores.

Symmetric-halving strategy: sim = sim^T, so each element's exp contributes to
exactly two row sums (rows i and j). Each core owns 2048 rows (inputs rotated
by -core*2048 so its rows are global columns [0, 2048)) and computes only the
upper-triangular part of its strip at 128x128 subblock granularity:

  range A: columns [m*128, 8192)          (own diag block tri + blocks 1..3)
  range B: columns [8192 + m*128, 10240)  (block 4 tri, split with core+4)

for each 128-row strip m. Row sums come from the ACT Exp pass (fused affine
exp(S*dot - S), S = 1/temperature, fixed shift since rows are unit norm);
the mirrored contributions are recovered as column sums of each exp'd
128-column chunk via near-free tiny matmuls (lhsT = exp chunk, rhs = ones,
moving dim 1). Blocks 5..7 are never touched: their contributions arrive as
column sums computed by cores me+5..7. Final assembly (adding column-sum
contributions into other rows' denominators, log, mean) happens on the host
in float64.

Scheduling: ACT is the bottleneck engine (~111us of Exp busy at 1 col/cycle
@1.2GHz), so the program keeps its in-order stream free of everything else:
rsqrt is computed on DVE (bit-trick seed + 2 Newton steps), pieces are
emitted piece-major (all strips' piece 0 first) so the first exp wave only
needs ET blocks 0-1, and ET blocks 2-4 are built mid-wave. Main matmul
bursts lead with a 128-wide matmul (PE p-state ramps from lowest on each
burst). Column-sum tiny matmuls for piece i are emitted after the main
matmuls of piece i+2 so PE's in-order stream never waits on ACT.

loss = 1/T + mean_i log(S_i) - mean_i dot(a_i, b_i)/T
"""

import sys

if "/opt/trn_rl_repo" not in sys.path:
    sys.path.insert(0, "/opt/trn_rl_repo")

import numpy as np

import concourse.bass as bass
import concourse.mybir as mybir
from concourse import masks
from concourse.tile import TileContext
from concourse.bass_utils import run_bass_kernel_spmd

# ---------------------------------------------------------------------------
# Compatibility patches for the walrus build in this container:
#  * EVENT_SEMAPHORE_RANGE_CLEAR fails codegen ("ISA wrong length"), and
#  * the tile teardown Drain carries >2 sem waits ("Too many sync wait
#    commands").
# Replace the teardown with per-proc single-wait drains + barriers and skip
# the on-device semaphore clear (allocator bookkeeping is kept).
# ---------------------------------------------------------------------------


def _patched_clear_and_free_semaphores(self, sems):
    if not sems:
        return
    sem_nums = [
        s.num if isinstance(s, bass.SemaphoreHandle) else s for s in sems
    ]
    self._state.prepend_free_semaphores(sem_nums)
    for poison_set in self._tile_sem_poison_stack:
        poison_set.update(sem_nums)


def _patched_drain_and_barrier(self, tick_clock, wait_clock):
    nc = self.nc
    clock = tick_clock.global_clock
    assert self.sems is not None
    allocated = self.sems.allocated()  # proc index -> SemaphoreHandle
    for proc in sorted(allocated):
        sem = allocated[proc]
        tick = clock[proc]
        if tick <= 0:
            continue
        mult = 16 if sem.name.startswith("DMA") else 1
        d = nc.sync.drain()
        d.wait_op(sem, tick * mult, "sem-ge")
    nc.all_engine_barrier()
    popped = nc._tile_sem_poison_stack.pop()
    assert popped is self._sem_poison
    nc.clear_and_free_semaphores(list(allocated.values()))
    nc.all_engine_barrier()


bass.Bass.clear_and_free_semaphores = _patched_clear_and_free_semaphores
TileContext._drain_and_barrier = _patched_drain_and_barrier


def _hoist_excess_waits(nc, limit=1):
    """This walrus supports only `limit` sync waits per instruction. Hoist
    the excess onto standalone EventSemaphore instructions inserted just
    before the over-subscribed instruction on the same engine (per-engine
    program order makes this semantically identical)."""
    import bass_rust

    counter = 0
    for bb in nc.main_func.blocks:
        insts = bb.instructions
        new = []
        changed = False
        for ins in insts:
            si = ins.sync_info
            if si is not None:
                waits = list(si.on_wait)
                if len(waits) > limit:
                    excess, keep = waits[:-limit], waits[-limit:]
                    for w in excess:
                        counter += 1
                        ev = mybir.InstEventSemaphore(
                            name=f"hoistw-{counter}",
                            engine=ins.engine,
                            ins=[],
                            outs=[],
                        )
                        ev.sync_info = bass_rust.SyncInfo(
                            on_wait=[w], on_update=[]
                        )
                        new.append(ev)
                    ins.sync_info = bass_rust.SyncInfo(
                        on_wait=keep, on_update=list(si.on_update)
                    )
                    changed = True
            new.append(ins)
        if changed:
            bb.instructions = new

TEMPERATURE = 0.07
B, D = 8192, 128
N2 = 2 * B
NCORES = 8
P = 128
RPC = N2 // NCORES      # 2048 rows per core
MT = RPC // P           # 16 row strips
BS = 2048               # block size (columns per block / rows per DMA block)
JB = BS // P            # 16 rows packed per partition
NBLK = 5                # et blocks needed: 0..4
PW = 1536               # psum piece width (3 banks)
NSLOT = 78              # column-sum slots: b0 15 + b1..3 48 + b4 15
MAXPIECES = 8           # per-strip piece slots in partials

F32 = mybir.dt.float32
I32 = mybir.dt.int32
BF16 = mybir.dt.bfloat16
AF = mybir.ActivationFunctionType
ALU = mybir.AluOpType
AX = mybir.AxisListType


def _slot_of(k, cc):
    """csacc slot for colsums of block k, chunk cc (None = no colsum)."""
    if k == 0:
        return cc - 1           # cc in 1..15 -> 0..14
    if k in (1, 2, 3):
        return 15 + (k - 1) * 16 + cc   # 15..62
    return 63 + cc - 1          # block 4, cc in 1..15 -> 63..77


def _strip_pieces(m):
    """Pieces for strip m: list of (g0, w, mask_diag, slot_chunks)."""
    pieces = []
    for lo, hi in ((m * 128, 8192), (8192 + m * 128, 10240)):
        pos = lo
        while pos < hi:
            w = min(PW, hi - pos)
            slot_chunks = []
            mask_diag = False
            for ci in range(w // 128):
                c_abs = (pos + ci * 128) // 128
                k, cc = c_abs // 16, c_abs % 16
                if cc == m and k in (0, 4):
                    if k == 0:
                        mask_diag = True
                        assert ci == 0
                    continue
                slot_chunks.append((ci, _slot_of(k, cc)))
            if slot_chunks:
                s0 = slot_chunks[0][1]
                assert [s for _, s in slot_chunks] == list(
                    range(s0, s0 + len(slot_chunks))
                )
            pieces.append((pos, w, mask_diag, slot_chunks))
            pos += w
    assert len(pieces) <= MAXPIECES
    return pieces


def _build_bass(hoist=True):
    scale = 1.0 / TEMPERATURE

    nc = bass.Bass()
    allx = nc.dram_tensor("allx", [NBLK * BS, D], BF16, kind="ExternalInput")
    out = nc.dram_tensor("out", [P, 128 + NSLOT + JB], F32,
                         kind="ExternalOutput")

    # packed view: row = b*BS + p*JB + j
    allx_b = allx[:].rearrange("(b p j) d -> b p (j d)", p=P, j=JB)

    with TileContext(nc) as tc:
        with (
            tc.tile_pool(name="persist", bufs=1) as persist,
            tc.tile_pool(name="raw0", bufs=1) as raw0_pool,
            tc.tile_pool(name="rawx", bufs=4) as rawx_pool,
            tc.tile_pool(name="xn", bufs=2) as xn_pool,
            tc.tile_pool(name="sq", bufs=3) as sq_pool,
            tc.tile_pool(name="exps", bufs=4) as exps_pool,
            tc.tile_pool(name="psum", bufs=2, space="PSUM") as psum_pool,
            tc.tile_pool(name="cspsum", bufs=1, space="PSUM") as cs_pool,
            tc.tile_pool(name="tpsum", bufs=1, space="PSUM") as tp_pool,
        ):
            ident = persist.tile([P, P], BF16, tag="ident")
            masks.make_identity(nc, ident[:])
            bias_negs = persist.tile([P, 1], F32, tag="bias_negs")
            nc.gpsimd.memset(bias_negs[:], -scale)
            ones_bf = persist.tile([P, 1], BF16, tag="ones_bf")
            nc.gpsimd.memset(ones_bf[:], 1.0)
            dmask = persist.tile([P, P], F32, tag="dmask")
            nc.gpsimd.memset(dmask[:], 0.0)
            nc.gpsimd.affine_select(
                out=dmask[:], in_=dmask[:],
                compare_op=ALU.not_equal, fill=-1.0e9,
                base=0, pattern=[[-1, P]], channel_multiplier=1,
            )

            et = persist.tile([P, NBLK * BS], BF16, tag="et")
            norms2 = persist.tile([P, NBLK * JB], F32, tag="norms2")
            rsq = persist.tile([P, NBLK * JB], F32, tag="rsq")
            rs_i = persist.tile([P, NBLK * JB], I32, tag="rs_i")
            rs_a = persist.tile([P, NBLK * JB], F32, tag="rs_a")
            rs_c = persist.tile([P, NBLK * JB], F32, tag="rs_c")
            rawdot = persist.tile([P, JB], F32, tag="rawdot")
            pos2 = persist.tile([P, JB], F32, tag="pos2")
            pospart = persist.tile([P, JB], F32, tag="pospart")
            partials = persist.tile([P, MT * MAXPIECES], F32, tag="partials")
            csacc = persist.tile([P, NSLOT], F32, tag="csacc")
            nc.gpsimd.memset(csacc[:], 0.0)

            # ---- DMAs up front ----
            raw_blocks = {}
            sq_tiles = {}
            for b in range(NBLK):
                pool = raw0_pool if b == 0 else rawx_pool
                rx = pool.tile([P, BS], BF16, tag="raw0" if b == 0 else "")
                nc.sync.dma_start(rx[:], allx_b[b])
                raw_blocks[b] = rx

            def build_block(b):
                """reduce + DVE rsqrt + normalize + transpose into et."""
                rx = raw_blocks[b]
                rx3 = rx[:].rearrange("p (j d) -> p j d", d=D)
                js = slice(b * JB, (b + 1) * JB)
                if b < 2:
                    sq = sq_pool.tile([P, BS], F32)
                    nc.vector.tensor_mul(sq[:], rx[:], rx[:])
                else:
                    sq = sq_tiles[b]
                nc.vector.reduce_sum(
                    norms2[:, js], sq[:].rearrange("p (j d) -> p j d", d=D),
                    axis=AX.X,
                )
                # rsqrt on DVE: bit-trick seed + 2 Newton iterations
                nsl = norms2[:, js]
                til = rs_i[:, js]
                aa = rs_a[:, js]
                cc = rs_c[:, js]
                yy = rsq[:, js]
                nc.vector.tensor_scalar(
                    til, nsl.bitcast(I32), 1, None,
                    op0=ALU.logical_shift_right,
                )
                nc.vector.tensor_scalar(
                    til, til, -1, 0x5F3759DF, op0=ALU.mult, op1=ALU.add
                )
                y0 = til.bitcast(F32)
                for it in range(2):
                    src = y0 if it == 0 else yy
                    nc.vector.tensor_mul(aa, src, src)
                    nc.vector.tensor_mul(aa, aa, nsl)
                    nc.vector.tensor_scalar(
                        cc, aa, -0.5, 1.5, op0=ALU.mult, op1=ALU.add
                    )
                    nc.vector.tensor_mul(yy, src, cc)
                xn = xn_pool.tile([P, BS], BF16)
                nc.vector.tensor_mul(
                    xn[:].rearrange("p (j d) -> p j d", d=D),
                    rx3,
                    rsq[:, js].to_broadcast((P, JB, D)),
                )
                xn3 = xn[:].rearrange("p (j d) -> p j d", d=D)
                etb = et[:, b * BS:(b + 1) * BS].rearrange(
                    "q (p j) -> q p j", j=JB
                )
                for h in range(2):  # two 8-transpose halves, 1 psum bank
                    ps = tp_pool.tile([P, 8 * P], BF16, tag="tp")
                    for j in range(8):
                        nc.tensor.transpose(
                            ps[:, j * P:(j + 1) * P],
                            xn3[:, h * 8 + j, :], ident[:],
                        )
                    nc.vector.tensor_copy(
                        etb[:, :, h * 8:(h + 1) * 8],
                        ps[:].rearrange("q (j p) -> q p j", p=P),
                    )

            def emit_pospair():
                r0 = raw_blocks[0][:].rearrange("p (j d) -> p j d", d=D)
                rp = raw_blocks[4][:].rearrange("p (j d) -> p j d", d=D)
                pd = sq_pool.tile([P, BS], F32)
                pd3 = pd[:].rearrange("p (j d) -> p j d", d=D)
                nc.vector.tensor_mul(pd3[:], r0[:], rp[:])
                nc.vector.reduce_sum(rawdot[:], pd3[:], axis=AX.X)
                nc.vector.tensor_mul(pos2[:], rawdot[:], rsq[:, 0:JB])
                nc.vector.tensor_mul(
                    pospart[:], pos2[:], rsq[:, 4 * JB:5 * JB]
                )

            build_block(0)
            build_block(1)
            # squares for blocks 2-4 on Pool (allocated after build 0/1's
            # sq tiles so pool-slot reuse never waits on later instructions)
            for b in range(2, NBLK):
                sq = sq_pool.tile([P, BS], F32)
                nc.gpsimd.tensor_mul(sq[:], raw_blocks[b][:], raw_blocks[b][:])
                sq_tiles[b] = sq

            # ---- main loop: piece-major waves, software-pipelined ----
            LAG = 2
            per_strip = [_strip_pieces(m) for m in range(MT)]
            schedule = []
            for pi in range(max(len(p) for p in per_strip)):
                for m in range(MT):
                    if pi < len(per_strip[m]):
                        schedule.append(("piece", m, pi))
                        if m == 7 and 0 <= pi <= 2:
                            schedule.append(("build", pi + 2))

            def emit_colsums(piece, ex):
                _m, _pi, _g0, _w, _md, slot_chunks = piece
                if not slot_chunks:
                    return
                cs = cs_pool.tile([P, JB], F32)
                s0 = slot_chunks[0][1]
                n = len(slot_chunks)
                for si, (ci, _slot) in enumerate(slot_chunks):
                    nc.tensor.matmul(
                        cs[:, si:si + 1],
                        ex[:, ci * P:(ci + 1) * P],
                        ones_bf[:],
                        start=True, stop=True,
                    )
                nc.vector.tensor_add(
                    csacc[:, s0:s0 + n], csacc[:, s0:s0 + n], cs[:, 0:n]
                )

            inflight = []
            for item in schedule:
                if item[0] == "build":
                    build_block(item[1])
                    if item[1] == 4:
                        emit_pospair()
                    continue
                _, m, pi = item
                g0, w, mask_diag, slot_chunks = per_strip[m][pi]
                piece = (m, pi, g0, w, mask_diag, slot_chunks)
                lhsT = et[:, m * P:(m + 1) * P]
                ps = psum_pool.tile([P, PW], F32, tag="ps")
                # lead with a short matmul: first instruction of a PE burst
                # issues at the lowest p-state
                cuts = [0, 128] + list(range(512, w, 512)) + [w]
                for c0, c1 in zip(cuts, cuts[1:]):
                    if c0 >= c1:
                        continue
                    nc.tensor.matmul(
                        ps[:, c0:c1], lhsT, et[:, g0 + c0:g0 + c1],
                        start=True, stop=True,
                    )
                if mask_diag:
                    nc.vector.tensor_add(ps[:, 0:P], ps[:, 0:P], dmask[:])
                ex = exps_pool.tile([P, PW], BF16, name="ex")
                pidx = m * MAXPIECES + pi
                nc.scalar.activation(
                    ex[:, 0:w], ps[:, 0:w], AF.Exp,
                    bias=bias_negs[:], scale=scale,
                    accum_out=partials[:, pidx:pidx + 1],
                )
                inflight.append((piece, ex))
                if len(inflight) > LAG:
                    emit_colsums(*inflight.pop(0))
            for pc_ex in inflight:
                emit_colsums(*pc_ex)

            # ---- outputs: raw partials, colsums, positive dots ----
            nc.sync.dma_start(out[:, 0:128], partials[:])
            nc.sync.dma_start(out[:, 128:128 + NSLOT], csacc[:])
            nc.sync.dma_start(out[:, 128 + NSLOT:], pospart[:])

    if hoist:
        _hoist_excess_waits(nc, limit=1)
    return nc


def _in_maps(embeddings_a, embeddings_b, ncores=NCORES):
    import ml_dtypes

    allx = np.ascontiguousarray(
        np.concatenate([embeddings_a, embeddings_b], axis=0)
    ).astype(ml_dtypes.bfloat16)
    maps = []
    for c in range(ncores):
        # rotate so this core's rows sit at columns [0, RPC); only blocks
        # 0..4 are needed (5..7 arrive as other cores' column sums)
        rot = np.ascontiguousarray(
            np.roll(allx, -c * RPC, axis=0)[: NBLK * BS]
        )
        maps.append({"allx": rot})
    return maps


def _combine(outs):
    """outs: per-core [P, 128+NSLOT+JB] raw partials -> scalar loss."""
    S = np.zeros(N2, dtype=np.float64)
    dots = 0.0
    prow = np.arange(P)
    for c, o in enumerate(outs):
        o64 = np.asarray(o, dtype=np.float64)
        r0 = c * RPC
        part = o64[:, 0:128]
        cso = o64[:, 128:128 + NSLOT]
        pos = o64[:, 128 + NSLOT:]
        # row-accum partial sums per strip
        for m in range(MT):
            npc = len(_strip_pieces(m))
            rows = (r0 + m * 128 + prow) % N2
            S[rows] += part[:, m * MAXPIECES:m * MAXPIECES + npc].sum(axis=1)
        # column-sum contributions
        for s in range(NSLOT):
            if s < 15:
                k, cc = 0, s + 1
            elif s < 63:
                k, cc = 1 + (s - 15) // 16, (s - 15) % 16
            else:
                k, cc = 4, s - 63 + 1
            rows = (r0 + k * BS + cc * 128 + prow) % N2
            S[rows] += cso[:, s]
        dots += pos.sum()
    inv_t = 1.0 / TEMPERATURE
    loss = inv_t + np.mean(np.log(S)) - (dots * inv_t) / N2
    return np.float32(loss)


_NC_CACHE = {}


def _get_nc():
    if "nc" not in _NC_CACHE:
        _NC_CACHE["nc"] = _build_bass()
    return _NC_CACHE["nc"]


def kernel(embeddings_a, embeddings_b):
    nc = _get_nc()
    maps = _in_maps(embeddings_a, embeddings_b)
    res = run_bass_kernel_spmd(nc, maps, list(range(NCORES)), trace=False)
    return _combine([r["out"] for r in res.results])
